# revision 2
# baseline (speedup 1.0000x reference)
"""MultiHeadAttentionPool3D on 8 Trainium2 NeuronCores.

Math (per batch b):
  scores[hq, s] = scale * (q_eff[hq, :] @ x[b, :, s])     (key-projection folded into
                                                           the queries; per-row bias
                                                           terms cancel in softmax)
  p = exp(scores)       (no max-subtraction: scores ~ N(0,1), fp32-safe)
  l[hq] = sum_s p[hq, s];   y[hq, c] = sum_s p[hq, s] * x[b, c, s]
  pooled = y / l  -> tiny epilogue (value proj, Wo, layernorm) on host.

Sharding: core = b * 2 + s_half  (4 batches x 2 halves of S=32768).

v10 design (v9 + fp8 storage for x):
  - x is staged in HBM as fp8e4m3 in BOTH layouts: x [C, S_loc] (c on
    partitions) and the flat transposed plane xt [128, n_sb*256] (s on
    partitions; column block j = x[:, j*128:(j+1)*128].T).  This halves
    HBM traffic (the kernel is DMA-bound) and halves PE LDWEIGHTS time
    (FWL reads 4 fp8/cycle vs 2 fp16).  q_eff and p stay fp16, so only
    x is quantized; the softmax smooths the key-side error and the
    value-side error averages out over ~10^4 effective samples.
  - scoresT per s-block via x-tile-as-stationary:
      psum_sT[:, sb*32:+32] += x_half[:, sb*128:+128].T @ q_effT_half
    16 s-blocks packed per [128, 512] PSUM bank; the output IS pT-oriented,
    so p never needs transposing.
  - p = Exp(scale*scoresT): one [128,512] ScalarE activation per chunk -> fp16.
  - l partials: ones[128,1].T @ pT_chunk -> psum_l [1, 512], accumulated
    across chunks (host sums the 16 slots).
  - y: per s-block and c-half: xt_slice[s,128].T @ pT_slice[s,32]
    -> psum_y [128(c-half), 64(2*hq)], PSUM-accumulated over all s.
  - host reassembles y/l and runs the tiny epilogue.
"""

import sys

if "/opt/trn_rl_repo" not in sys.path:
    sys.path.insert(0, "/opt/trn_rl_repo")

import numpy as np

NUM_HEADS = 8
OUT_FEATURES = 512
NUM_QUERIES = 4
C = 256
HEAD_DIM = OUT_FEATURES // NUM_HEADS
LN_EPS = 1e-5
B = 4
S = 32 * 32 * 32
N_CORES = 8
S_LOC = S // 2  # shard: (batch, half of spatial axis)
HQ = NUM_HEADS * NUM_QUERIES  # 32 fused query rows, hq = h*NUM_QUERIES + q
SCALE = HEAD_DIM ** -0.5
CHUNK = 2048

_NC_CACHE = {}


def _np_f8():
    import ml_dtypes

    return ml_dtypes.float8_e4m3


def _build_nc(s_loc=S_LOC, chunk=CHUNK, loop_n=1, x_f8=True, multi_queue=True):
    import concourse.bass as bass
    import concourse.tile as tile
    from concourse import bacc, mybir
    import contextlib

    f32 = mybir.dt.float32
    f16 = mybir.dt.float16
    xdt = mybir.dt.float8e4 if x_f8 else f16
    Exp = mybir.ActivationFunctionType.Exp

    if isinstance(chunk, int):
        assert s_loc % chunk == 0 and chunk % 512 == 0
        sizes = [chunk] * (s_loc // chunk)
    else:
        sizes = list(chunk)
        assert sum(sizes) == s_loc and all(c % 512 == 0 for c in sizes)
    assert sizes[0] >= 2048, "first chunk must init the full l zero-region"
    n_ch = len(sizes)
    n_sb = s_loc // 128
    W = 256

    nc = bacc.Bacc("TRN2", target_bir_lowering=False, debug=False,
                   num_devices=N_CORES)
    x_d = nc.dram_tensor("x", [C, s_loc], xdt, kind="ExternalInput")
    xt_d = nc.dram_tensor("xt", [128, n_sb * W], xdt, kind="ExternalInput")
    qT_d = nc.dram_tensor("qT", [C, HQ], f16, kind="ExternalInput")
    y_d = nc.dram_tensor("y", [128, 2 * HQ], f32, kind="ExternalOutput")
    l_d = nc.dram_tensor("l", [1, 512], f32, kind="ExternalOutput")

    with tile.TileContext(nc) as tc:
        with (
            tc.tile_pool(name="const", bufs=1) as constp,
            tc.tile_pool(name="xstage", bufs=3) as xstage,
            tc.tile_pool(name="xtstage", bufs=3) as xtstage,
            tc.tile_pool(name="ptstage", bufs=3) as ptstage,
            tc.tile_pool(name="outp", bufs=2) as outp,
            tc.tile_pool(name="ps_st", bufs=3, space="PSUM") as ps_st,
            tc.tile_pool(name="ps_y", bufs=1, space="PSUM") as ps_yp,
            tc.tile_pool(name="ps_l", bufs=1, space="PSUM") as ps_lp,
        ):
            qt0 = constp.tile([128, HQ], f16)
            nc.sync.dma_start(qt0[:], qT_d[0:128, :])
            qt1 = constp.tile([128, HQ], f16)
            nc.sync.dma_start(qt1[:], qT_d[128:256, :])
            ones = constp.tile([128, 1], f16)
            nc.gpsimd.memset(ones[:], 1.0)

            def iter_scope():
                if loop_n > 1:
                    E = mybir.EngineType
                    return tc.For_i(0, loop_n, 1,
                                    hint_engines=(E.PE, E.DVE, E.Activation,
                                                  E.SP, E.Pool))
                return contextlib.nullcontext()

            q0 = nc.sync
            q1 = nc.scalar if multi_queue else nc.sync
            with iter_scope():
                psum_y = ps_yp.tile([128, 2 * HQ], f32, tag="psy")
                psum_l = ps_lp.tile([1, 512], f32, tag="psl")

                offs = [sum(sizes[:i]) for i in range(n_ch)]
                for k in range(n_ch):
                    o = offs[k]
                    ck = sizes[k]
                    sbpc = ck // 128
                    xc0 = xstage.tile([128, ck], xdt, tag="xc0")
                    q0.dma_start(xc0[:], x_d[0:128, o:o + ck])
                    xc1 = xstage.tile([128, ck], xdt, tag="xc1")
                    q0.dma_start(xc1[:], x_d[128:256, o:o + ck])
                    xt_c = xtstage.tile([128, sbpc * W], xdt, tag="xt")
                    ocol = (o // 128) * W
                    q1.dma_start(xt_c[:], xt_d[:, ocol:ocol + sbpc * W])

                    # scoresT: 16 s-blocks packed into one [128, 512] bank
                    pst = ps_st.tile([128, sbpc * HQ], f32, tag="pst")
                    for sb in range(sbpc):
                        for h, (xc, qt) in enumerate(((xc0, qt0), (xc1, qt1))):
                            nc.tensor.matmul(
                                pst[:, sb * HQ:(sb + 1) * HQ],
                                xc[:, sb * 128:(sb + 1) * 128], qt[:],
                                start=(sb == 0 and h == 0),
                                stop=(sb == sbpc - 1 and h == 1),
                                skip_group_check=True)
                    pt_c = ptstage.tile([128, sbpc * HQ], f16, tag="pt")
                    nc.scalar.activation(pt_c[:], pst[:], Exp, scale=SCALE)
                    # l partials: sum over the 128 s-rows of this chunk
                    ngr = (sbpc + 15) // 16
                    for g in range(ngr):
                        wsb = min(16, sbpc - g * 16)
                        nc.tensor.matmul(
                            psum_l[:, 0:wsb * HQ], ones[:],
                            pt_c[:, g * 16 * HQ:(g * 16 + wsb) * HQ],
                            start=(k == 0 and g == 0),
                            stop=(k == n_ch - 1 and g == ngr - 1),
                            skip_group_check=True)
                    # y: [c-half, hq] accumulated over all s-blocks
                    for sb in range(sbpc):
                        gsb = o // 128 + sb
                        for h in (0, 1):
                            nc.tensor.matmul(
                                psum_y[:, h * HQ:(h + 1) * HQ],
                                xt_c[:, sb * W + h * 128:sb * W + (h + 1) * 128],
                                pt_c[:, sb * HQ:(sb + 1) * HQ],
                                start=(gsb == 0 and h == 0),
                                stop=(gsb == n_sb - 1 and h == 1),
                                skip_group_check=True)

                y_t = outp.tile([128, 2 * HQ], f32, tag="yt")
                nc.vector.tensor_copy(y_t[:], psum_y[:])
                l_t = outp.tile([1, 512], f32, tag="lt")
                nc.vector.tensor_copy(l_t[:], psum_l[:])
                nc.sync.dma_start(y_d[:], y_t[:])
                nc.sync.dma_start(l_d[:], l_t[:])

    nc.compile()
    return nc


def _get_nc(loop_n=1, x_f8=True, chunk=CHUNK, multi_queue=True):
    key = (S_LOC, loop_n, x_f8, chunk, multi_queue)
    if key not in _NC_CACHE:
        _NC_CACHE[key] = _build_nc(loop_n=loop_n, x_f8=x_f8, chunk=chunk,
                                   multi_queue=multi_queue)
    return _NC_CACHE[key]


def _shard_inputs(shard, qT, s_loc=S_LOC, x_f8=True):
    """shard: [C, s_loc] fp32 -> in_map for one core."""
    n_sb = s_loc // 128
    xdt = _np_f8() if x_f8 else np.float16
    x8 = shard.astype(xdt)
    # flat transposed plane: column block j (width C) = shard[:, j*128:+128].T
    xt = np.ascontiguousarray(
        shard.T.reshape(n_sb, 128, C).astype(xdt)
        .transpose(1, 0, 2).reshape(128, n_sb * C))
    return {"x": np.ascontiguousarray(x8), "xt": xt,
            "qT": qT.astype(np.float16)}


def _prepare_in_maps(x, queries, Wk, x_f8=True):
    xf = np.ascontiguousarray(np.asarray(x, np.float32).reshape(B, C, S))
    qr = np.asarray(queries, np.float32).reshape(NUM_QUERIES, NUM_HEADS, HEAD_DIM)
    Wkr = np.asarray(Wk, np.float32).reshape(NUM_HEADS, HEAD_DIM, C)
    # q_eff[h*NQ+q, c] = sum_d q[q,h,d] * Wk[h*hd+d, c]
    q_eff = np.einsum("qhd,hdc->hqc", qr, Wkr).reshape(HQ, C)
    qT = np.ascontiguousarray(q_eff.T.astype(np.float32))
    in_maps = []
    for core in range(N_CORES):
        b, half = divmod(core, 2)
        shard = np.ascontiguousarray(xf[b, :, half * S_LOC:(half + 1) * S_LOC])
        in_maps.append(_shard_inputs(shard, qT, x_f8=x_f8))
    return in_maps


def _extract_yl(yv, lv):
    """Device outputs -> (Y [HQ, C], L [HQ]) for one core."""
    Y = np.concatenate([yv[:, 0:HQ].T, yv[:, HQ:2 * HQ].T], axis=1)  # [HQ, 256]
    L = lv.reshape(-1, HQ).sum(axis=0)
    return Y, L


def _epilogue(Y, L, Wv, bv, Wo, bo, gamma, beta):
    """Y [B, HQ, C], L [B, HQ] -> final [B, OUT_FEATURES]."""
    pooled = (Y / L[:, :, None]).reshape(B, NUM_HEADS, NUM_QUERIES, C)
    Wvr = np.asarray(Wv, np.float32).reshape(NUM_HEADS, HEAD_DIM, C)
    att = np.einsum("hdc,bhqc->bhqd", Wvr, pooled)
    att += np.asarray(bv, np.float32).reshape(1, NUM_HEADS, 1, HEAD_DIM)
    multi = att.transpose(0, 2, 1, 3).reshape(B, NUM_QUERIES * OUT_FEATURES)
    out = multi @ np.asarray(Wo, np.float32).T + np.asarray(bo, np.float32)
    mu = out.mean(-1, keepdims=True)
    var = ((out - mu) ** 2).mean(-1, keepdims=True)
    out = (out - mu) / np.sqrt(var + LN_EPS)
    out = out * np.asarray(gamma, np.float32) + np.asarray(beta, np.float32)
    return out.astype(np.float32)


def kernel(x, queries, Wk, bk, Wv, bv, Wo, bo, gamma, beta):
    from concourse.bass_utils import run_bass_kernel_spmd

    in_maps = _prepare_in_maps(x, queries, Wk)
    nc = _get_nc()
    res = run_bass_kernel_spmd(nc, in_maps, list(range(N_CORES))).results
    Y = np.zeros((B, HQ, C), np.float32)
    L = np.zeros((B, HQ), np.float32)
    for core in range(N_CORES):
        b = core // 2
        Yc, Lc = _extract_yl(res[core]["y"], res[core]["l"])
        Y[b] += Yc
        L[b] += Lc
    return _epilogue(Y, L, Wv, bv, Wo, bo, gamma, beta)


# revision 51
# speedup vs baseline: 2.5707x; 2.5707x over previous
"""MultiHeadAttentionPool3D on 8 Trainium2 NeuronCores.

Math (per batch b):
  scores[hq, s] = scale * (q_eff[hq, :] @ x[b, :, s])     (key-projection folded into
                                                           the queries; per-row bias
                                                           terms cancel in softmax)
  p = exp(scores)       (no max-subtraction: scores ~ N(0,1), fp32-safe)
  l[hq] = sum_s p[hq, s];   y[hq, c] = sum_s p[hq, s] * x[b, c, s]
  pooled = y / l  -> tiny epilogue (value proj, Wo, layernorm) on host.

Sharding: core = b * 2 + s_half  (4 batches x 2 halves of S=32768).

v11 design (fp8 storage for x + software-pipelined y):
  - x is staged in HBM as fp8e4m3 in BOTH layouts: x [C, S_loc] (c on
    partitions) and the flat transposed plane xt [128, n_sb*256] (s on
    partitions; column block j = x[:, j*128:(j+1)*128].T).  This halves
    HBM traffic (the kernel is DMA-bound) and halves PE LDWEIGHTS time
    (FWL reads 4 fp8/cycle vs 2 fp16).  q_eff and p stay fp16, so only
    x is quantized; the softmax smooths the key-side error and the
    value-side error averages out over ~10^4 effective samples.
  - scoresT per s-block via x-tile-as-stationary:
      psum_sT[:, sb*32:+32] += x_half[:, sb*128:+128].T @ q_effT_half
    16 s-blocks packed per [128, 512] PSUM bank; the output IS pT-oriented,
    so p never needs transposing.
  - p = Exp(scale*scoresT): one [128,512] ScalarE activation per chunk -> fp16.
  - l partials: ones[128,1].T @ pT_chunk -> psum_l [1, 512], accumulated
    across chunks (host sums the 16 slots).
  - y: per s-block and c-half: xt_slice[s,128].T @ pT_slice[s,32]
    -> psum_y [128(c-half), 64(2*hq)], PSUM-accumulated over all s.
    Each chunk's l/y matmuls are emitted AFTER the next chunk's scores
    (pipe_y) so the in-order PE queue never stalls waiting for the Exp.
  - host reassembles y/l and runs the tiny epilogue.

Measured on the 8-core axon pod: 36.2us/iter (baseline fp16 two-layout:
56.2us), rel err 5.9e-3.  Probes: DMA floor ~27us (HBM ~295-350GB/s/core
+ ~3us loop skeleton), PE ~87ns per self-loading matmul (LDWEIGHTS of the
128-col x stationary does not overlap the matmuls; K=128 spans all PE
row-groups so the hardware pull-ahead never fires).  Rejected by
measurement: super-chunk DMAs, gpsimd third queue, col-packed pt-stationary
y (tile_position), folding l into y via a ones-column in xt, DoubleRow
scores (slower AND rel err 2.3e-2), 1024/512-col chunks.
"""

import sys

if "/opt/trn_rl_repo" not in sys.path:
    sys.path.insert(0, "/opt/trn_rl_repo")

import numpy as np

NUM_HEADS = 8
OUT_FEATURES = 512
NUM_QUERIES = 4
C = 256
HEAD_DIM = OUT_FEATURES // NUM_HEADS
LN_EPS = 1e-5
B = 4
S = 32 * 32 * 32
N_CORES = 8
S_LOC = S // 2  # shard: (batch, half of spatial axis)
HQ = NUM_HEADS * NUM_QUERIES  # 32 fused query rows, hq = h*NUM_QUERIES + q
SCALE = HEAD_DIM ** -0.5
CHUNK = 2048

_NC_CACHE = {}


def _np_f8():
    import ml_dtypes

    return ml_dtypes.float8_e4m3


def _build_nc(s_loc=S_LOC, chunk=CHUNK, loop_n=1, x_f8=True, multi_queue=True,
              pipe_y=True, three_queue=False, dbg_skip=(), super_chunk=None,
              y_mode="xt_stat", scores_dr=False):
    import concourse.bass as bass
    import concourse.tile as tile
    from concourse import bacc, mybir
    import contextlib

    f32 = mybir.dt.float32
    f16 = mybir.dt.float16
    xdt = mybir.dt.float8e4 if x_f8 else f16
    Exp = mybir.ActivationFunctionType.Exp

    assert s_loc % chunk == 0 and chunk % 512 == 0
    assert chunk >= 2048 or y_mode == "pt_stat4_l", \
        "chunks must init the full l zero-region unless l is folded into y"
    sc = super_chunk or chunk
    assert s_loc % sc == 0 and sc % chunk == 0
    n_super = s_loc // sc
    n_inner = sc // chunk
    n_ch = s_loc // chunk
    n_sb = s_loc // 128
    # pt_stat4_l: xt blocks carry a trailing host-baked ones column, so the
    # y matmuls also produce the l partials (out col 256) and the separate
    # l matmuls disappear.
    W = 257 if y_mode == "pt_stat4_l" else 256

    nc = bacc.Bacc("TRN2", target_bir_lowering=False, debug=False,
                   num_devices=N_CORES)
    if scores_dr:
        # DoubleRow layout: x[p, sb*256 + ki*128 + m] = xf[ki*128+p, sb*128+m]
        assert x_f8, "DoubleRow needs fp8"
        x_d = nc.dram_tensor("x", [128, n_sb * 256], xdt, kind="ExternalInput")
        qT_d = nc.dram_tensor("qT", [128, 2 * HQ], xdt, kind="ExternalInput")
    else:
        x_d = nc.dram_tensor("x", [C, s_loc], xdt, kind="ExternalInput")
        qT_d = nc.dram_tensor("qT", [C, HQ], f16, kind="ExternalInput")
    xt_d = nc.dram_tensor("xt", [128, n_sb * W], xdt, kind="ExternalInput")
    y_cols = 2 * HQ if y_mode == "xt_stat" else W
    fold_l = y_mode == "pt_stat4_l"
    y_d = nc.dram_tensor("y", [128, y_cols], f32, kind="ExternalOutput")
    l_d = None if fold_l else nc.dram_tensor("l", [1, 512], f32,
                                             kind="ExternalOutput")

    with tile.TileContext(nc) as tc:
        xbufs = 2 if sc >= 16384 else 3
        with (
            tc.tile_pool(name="const", bufs=1) as constp,
            tc.tile_pool(name="xstage", bufs=xbufs) as xstage,
            tc.tile_pool(name="xtstage", bufs=xbufs) as xtstage,
            tc.tile_pool(name="ptstage", bufs=3) as ptstage,
            tc.tile_pool(name="outp", bufs=2) as outp,
            tc.tile_pool(name="ps_st", bufs=3, space="PSUM") as ps_st,
            tc.tile_pool(name="ps_y", bufs=2, space="PSUM") as ps_yp,
            tc.tile_pool(name="ps_l", bufs=2, space="PSUM") as ps_lp,
        ):
            if scores_dr:
                qtd = constp.tile([128, 2, HQ], xdt)
                nc.sync.dma_start(qtd[:, 0, :], qT_d[:, 0:HQ])
                nc.sync.dma_start(qtd[:, 1, :], qT_d[:, HQ:2 * HQ])
            else:
                qt0 = constp.tile([128, HQ], f16)
                nc.sync.dma_start(qt0[:], qT_d[0:128, :])
                qt1 = constp.tile([128, HQ], f16)
                nc.sync.dma_start(qt1[:], qT_d[128:256, :])
            ones = constp.tile([128, 1], f16)
            nc.gpsimd.memset(ones[:], 1.0)

            def iter_scope():
                if loop_n > 1:
                    E = mybir.EngineType
                    return tc.For_i(0, loop_n, 1,
                                    hint_engines=(E.PE, E.DVE, E.Activation,
                                                  E.SP, E.Pool))
                return contextlib.nullcontext()

            q0 = nc.sync
            q1 = nc.scalar if multi_queue else nc.sync
            q2 = nc.gpsimd if three_queue else q0
            dbg_skip = set(dbg_skip)
            with iter_scope():
                do_ly = "ly" not in dbg_skip
                if do_ly:
                    psum_y = ps_yp.tile([128, y_cols], f32, tag="psy")
                    if not fold_l:
                        psum_l = ps_lp.tile([1, 512], f32, tag="psl")

                sbpc = chunk // 128

                def emit_ly(k, xt_s, xto, pt_c):
                    if not fold_l:
                        # l partials: sum over the 128 s-rows of chunk k
                        ngr = (sbpc + 15) // 16
                        for g in range(ngr):
                            wsb = min(16, sbpc - g * 16)
                            nc.tensor.matmul(
                                psum_l[:, 0:wsb * HQ], ones[:],
                                pt_c[:, g * 16 * HQ:(g * 16 + wsb) * HQ],
                                start=(k == 0 and g == 0),
                                stop=(k == n_ch - 1 and g == ngr - 1),
                                skip_group_check=True)
                    if y_mode == "xt_stat":
                        # y: [c-half, hq] accumulated over all s-blocks
                        for sb in range(sbpc):
                            gsb = k * sbpc + sb
                            so = xto + sb * W
                            for h in (0, 1):
                                nc.tensor.matmul(
                                    psum_y[:, h * HQ:(h + 1) * HQ],
                                    xt_s[:, so + h * 128:so + (h + 1) * 128],
                                    pt_c[:, sb * HQ:(sb + 1) * HQ],
                                    start=(gsb == 0 and h == 0),
                                    stop=(gsb == n_sb - 1 and h == 1),
                                    skip_group_check=True)
                    else:
                        # yT: 4 col-group accumulators [32hq, 256c], s-block
                        # class j = gsb%4 at tile_position (0, 32j); pt is
                        # the 32-col stationary so 4 MMs run concurrently
                        for sb in range(sbpc):
                            gsb = k * sbpc + sb
                            so = xto + sb * W
                            j = gsb % 4
                            nc.tensor.matmul(
                                psum_y[32 * j:32 * (j + 1), :],
                                pt_c[:, sb * HQ:(sb + 1) * HQ],
                                xt_s[:, so:so + W],
                                start=(gsb < 4),
                                stop=(gsb >= n_sb - 4),
                                tile_position=(0, 32 * j),
                                skip_group_check=True)

                pending = None  # deferred (k, xt_s, xto, pt_c) for l/y
                for ks in range(n_super):
                    so = ks * sc
                    if "xdma" not in dbg_skip:
                        if scores_dr:
                            xcd = xstage.tile([128, (sc // 128) * 256], xdt,
                                              tag="xcd")
                            dcol = (so // 128) * 256
                            q0.dma_start(
                                xcd[:], x_d[:, dcol:dcol + (sc // 128) * 256])
                        else:
                            xc0 = xstage.tile([128, sc], xdt, tag="xc0")
                            xc1 = xstage.tile([128, sc], xdt, tag="xc1")
                            q0.dma_start(xc0[:], x_d[0:128, so:so + sc])
                            q2.dma_start(xc1[:], x_d[128:256, so:so + sc])
                    if "xtdma" not in dbg_skip:
                        xt_s = xtstage.tile([128, (sc // 128) * W], xdt,
                                            tag="xt")
                        ocol = (so // 128) * W
                        q1.dma_start(xt_s[:], xt_d[:, ocol:ocol + (sc // 128) * W])

                    for ki in range(n_inner):
                        k = ks * n_inner + ki
                        co = ki * chunk       # col offset into xc super tiles
                        xto = ki * sbpc * W   # col offset into xt super tile

                        # scoresT: 16 s-blocks packed per [128, 512] bank
                        do_scores = ("scores" not in dbg_skip
                                     and "xdma" not in dbg_skip)
                        if do_scores:
                            pst = ps_st.tile([128, sbpc * HQ], f32, tag="pst")
                            if scores_dr:
                                xv = xcd[:].rearrange(
                                    "p (sb k m) -> p sb k m", k=2, m=128)
                                for sb in range(sbpc):
                                    nc.tensor.matmul(
                                        pst[:, sb * HQ:(sb + 1) * HQ],
                                        xv[:, ki * sbpc + sb], qtd[:],
                                        perf_mode=mybir.MatmulPerfMode.DoubleRow,
                                        start=True, stop=True,
                                        skip_group_check=True)
                            else:
                                for sb in range(sbpc):
                                    for h, (xc, qt) in enumerate(((xc0, qt0),
                                                                  (xc1, qt1))):
                                        nc.tensor.matmul(
                                            pst[:, sb * HQ:(sb + 1) * HQ],
                                            xc[:, co + sb * 128:co + (sb + 1) * 128],
                                            qt[:],
                                            start=(sb == 0 and h == 0),
                                            stop=(sb == sbpc - 1 and h == 1),
                                            skip_group_check=True)
                            pt_c = ptstage.tile([128, sbpc * HQ], f16,
                                                tag="pt")
                            nc.scalar.activation(pt_c[:], pst[:], Exp,
                                                 scale=SCALE)
                        if not do_ly:
                            continue
                        if not do_scores:
                            pt_c = ptstage.tile([128, sbpc * HQ], f16,
                                                tag="pt")
                            nc.vector.memset(pt_c[:], 1.0)
                        if pipe_y:
                            # defer this chunk's l/y behind the NEXT chunk's
                            # scores so the PE never waits on the Exp
                            if pending is not None:
                                emit_ly(*pending)
                            pending = (k, xt_s, xto, pt_c)
                        else:
                            emit_ly(k, xt_s, xto, pt_c)
                if pending is not None:
                    emit_ly(*pending)

                y_t = outp.tile([128, y_cols], f32, tag="yt")
                if do_ly:
                    nc.vector.tensor_copy(y_t[:], psum_y[:])
                else:
                    nc.vector.memset(y_t[:], 0.0)
                nc.sync.dma_start(y_d[:], y_t[:])
                if not fold_l:
                    l_t = outp.tile([1, 512], f32, tag="lt")
                    if do_ly:
                        nc.vector.tensor_copy(l_t[:], psum_l[:])
                    else:
                        nc.vector.memset(l_t[:], 1.0)
                    nc.sync.dma_start(l_d[:], l_t[:])

    nc.compile()
    return nc


def _get_nc(loop_n=1, x_f8=True, chunk=CHUNK, multi_queue=True, pipe_y=True,
            three_queue=False, dbg_skip=(), super_chunk=None,
            y_mode="xt_stat", scores_dr=False):
    key = (S_LOC, loop_n, x_f8, chunk, multi_queue, pipe_y, three_queue,
           tuple(dbg_skip), super_chunk, y_mode, scores_dr)
    if key not in _NC_CACHE:
        _NC_CACHE[key] = _build_nc(loop_n=loop_n, x_f8=x_f8, chunk=chunk,
                                   multi_queue=multi_queue, pipe_y=pipe_y,
                                   three_queue=three_queue, dbg_skip=dbg_skip,
                                   super_chunk=super_chunk, y_mode=y_mode,
                                   scores_dr=scores_dr)
    return _NC_CACHE[key]


def _shard_inputs(shard, qT, s_loc=S_LOC, x_f8=True, xt_ones=False,
                  scores_dr=False):
    """shard: [C, s_loc] fp32 -> in_map for one core."""
    n_sb = s_loc // 128
    xdt = _np_f8() if x_f8 else np.float16
    if scores_dr:
        # x[p, sb*256 + ki*128 + m] = shard[ki*128+p, sb*128+m], fp8
        x8 = np.ascontiguousarray(
            shard.reshape(2, 128, n_sb, 128).transpose(1, 2, 0, 3)
            .reshape(128, n_sb * 256).astype(xdt))
    else:
        x8 = shard.astype(xdt)
    # flat transposed plane: column block j = shard[:, j*128:+128].T, plus an
    # optional trailing ones column per block (folds l into the y matmuls)
    W = C + 1 if xt_ones else C
    blocks = shard.T.reshape(n_sb, 128, C).astype(xdt).transpose(1, 0, 2)
    if xt_ones:
        aug = np.ones((128, n_sb, 1), xdt)
        blocks = np.concatenate([blocks, aug], axis=2)
    xt = np.ascontiguousarray(blocks.reshape(128, n_sb * W))
    if scores_dr:
        # qT here is q_eff [HQ, C] fp32 -> [128, 2*HQ] fp8 interleaved
        qd = np.ascontiguousarray(
            qT.T.reshape(HQ, 2, 128).transpose(2, 1, 0)
            .reshape(128, 2 * HQ).astype(xdt))
        return {"x": np.ascontiguousarray(x8), "xt": xt, "qT": qd}
    return {"x": np.ascontiguousarray(x8), "xt": xt,
            "qT": qT.astype(np.float16)}


def _prepare_in_maps(x, queries, Wk, x_f8=True, xt_ones=False,
                     scores_dr=False):
    xf = np.ascontiguousarray(np.asarray(x, np.float32).reshape(B, C, S))
    qr = np.asarray(queries, np.float32).reshape(NUM_QUERIES, NUM_HEADS, HEAD_DIM)
    Wkr = np.asarray(Wk, np.float32).reshape(NUM_HEADS, HEAD_DIM, C)
    # q_eff[h*NQ+q, c] = sum_d q[q,h,d] * Wk[h*hd+d, c]
    q_eff = np.einsum("qhd,hdc->hqc", qr, Wkr).reshape(HQ, C)
    qT = np.ascontiguousarray(q_eff.T.astype(np.float32))
    in_maps = []
    for core in range(N_CORES):
        b, half = divmod(core, 2)
        shard = np.ascontiguousarray(xf[b, :, half * S_LOC:(half + 1) * S_LOC])
        in_maps.append(_shard_inputs(shard, qT, x_f8=x_f8, xt_ones=xt_ones,
                                     scores_dr=scores_dr))
    return in_maps


def _extract_yl(yv, lv=None):
    """Device outputs -> (Y [HQ, C], L [HQ]) for one core."""
    if yv.shape[1] == 2 * HQ:  # xt_stat: [128c-half, 2*HQ]
        Y = np.concatenate([yv[:, 0:HQ].T, yv[:, HQ:2 * HQ].T], axis=1)
    elif yv.shape[1] == C:  # pt_stat4: sum the 4 col-group accumulators
        Y = yv.reshape(4, HQ, C).sum(axis=0)
    else:  # pt_stat4_l: [4*32hq, 257]; col 256 holds the l partials
        acc = yv.reshape(4, HQ, C + 1).sum(axis=0)
        return acc[:, :C], acc[:, C]
    L = lv.reshape(-1, HQ).sum(axis=0)
    return Y, L


def _epilogue(Y, L, Wv, bv, Wo, bo, gamma, beta):
    """Y [B, HQ, C], L [B, HQ] -> final [B, OUT_FEATURES]."""
    pooled = (Y / L[:, :, None]).reshape(B, NUM_HEADS, NUM_QUERIES, C)
    Wvr = np.asarray(Wv, np.float32).reshape(NUM_HEADS, HEAD_DIM, C)
    att = np.einsum("hdc,bhqc->bhqd", Wvr, pooled)
    att += np.asarray(bv, np.float32).reshape(1, NUM_HEADS, 1, HEAD_DIM)
    multi = att.transpose(0, 2, 1, 3).reshape(B, NUM_QUERIES * OUT_FEATURES)
    out = multi @ np.asarray(Wo, np.float32).T + np.asarray(bo, np.float32)
    mu = out.mean(-1, keepdims=True)
    var = ((out - mu) ** 2).mean(-1, keepdims=True)
    out = (out - mu) / np.sqrt(var + LN_EPS)
    out = out * np.asarray(gamma, np.float32) + np.asarray(beta, np.float32)
    return out.astype(np.float32)


# frozen best configuration (v11-pipe: fp8 x in both layouts, deferred y/l
# emission, 2048-column chunks, two HWDGE DMA queues)
BEST = {"x_f8": True, "pipe_y": True, "chunk": 2048, "y_mode": "xt_stat"}


def _best_in_maps(x, queries, Wk):
    return _prepare_in_maps(
        x, queries, Wk, x_f8=BEST["x_f8"],
        xt_ones=(BEST["y_mode"] == "pt_stat4_l"))


def _best_nc(loop_n=1):
    return _get_nc(loop_n=loop_n, **BEST)


def kernel(x, queries, Wk, bk, Wv, bv, Wo, bo, gamma, beta):
    from concourse.bass_utils import run_bass_kernel_spmd

    in_maps = _best_in_maps(x, queries, Wk)
    nc = _best_nc()
    res = run_bass_kernel_spmd(nc, in_maps, list(range(N_CORES))).results
    Y = np.zeros((B, HQ, C), np.float32)
    L = np.zeros((B, HQ), np.float32)
    for core in range(N_CORES):
        b = core // 2
        Yc, Lc = _extract_yl(res[core]["y"], res[core].get("l"))
        Y[b] += Yc
        L[b] += Lc
    return _epilogue(Y, L, Wv, bv, Wo, bo, gamma, beta)
